# revision 20
# baseline (speedup 1.0000x reference)
"""Trainium2 Bass kernel for temporal-mask multi-head attention.

Reference computation (per batch item b of 1024):
  q = x @ Wq + bq ; k = x @ Wk + bk ; v = x @ Wv + bv      (x: [176, 256])
  per head h (8 heads, dim 32):
    s = q_h @ k_h^T / sqrt(32)
    s = s*mask + (1-mask)*(-9e15)   (mask in {0,1}, symmetric)
    a = softmax(s, axis=-1) ; out_h = a @ v_h
  out = concat_h(out_h)   -> [1024, 176, 256]

Strategy (8 cores, batch-sharded 128 items/core):
  Host: pre-transpose x -> xT [B, 256, 176]; fold 1/sqrt(32) into Wq,bq;
        mask cast to bf16. Device computes, per item and head, the
        UNNORMALIZED attention output transposed (outT[d, sq]) plus the
        softmax denominator S[sq], via an augmented V-matmul whose
        stationary operand is [V_h | ones] (M=64; rows 32..63 all equal S).
        Host divides by S, adds bv, and transposes back.
  Device layouts (per item):
    XT   [256, 176]   (2 sbuf tiles [128, 176], f32, from host xT)
    QT,KT[256, 176]   = W^T X^T   (PE, weights stationary; + bias on DVE)
    V    [176, 256]   natural     (PE, lhsT = XT slices) -> bf16 sbuf
    sT_h [sk, sq]     = K_h^T-slices x Q_h  (PE, K=32 row-tiles, f32)
    e    = exp(sT)    (ACT, psum->sbuf bf16)
    em   = e * mask   (DVE for sk 0:128, GPSIMD for sk 128:176, bf16)
    outT_h_aug [64, 176] += [V_h|1]^T @ em_h   (PE, accumulated over sk)
    out dram [n, 2, 33, 4, 176] f32: rows 0:33 / 64:97 of the 4 psum
        pair-tiles (heads 2p / 2p+1), row 32 of each 33-block = S.
"""

import math
import os
import sys
from contextlib import ExitStack

import numpy as np

sys.path.insert(0, "/opt/trn_rl_repo")

import concourse.bass as bass  # noqa: E402
import concourse.bacc as bacc  # noqa: E402
import concourse.tile as tile  # noqa: E402
from concourse import mybir  # noqa: E402

F32 = mybir.dt.float32
BF16 = mybir.dt.bfloat16

SEQ = 176
D = 256
HEADS = 8
HD = 32
N_CORES = 8
BATCH = 1024


def build_nc(n_items: int, stage: int = 4, no_gps: bool = False,
             no_exp: bool = False, max_j: int = 4, j_list=None):
    """Build the per-core SPMD program processing n_items batch items.

    stage: 1=projections only, 2=+V, 3=+scores/exp/mask, 4=full.
    """
    assert n_items % 2 == 0
    G = 2  # items per projection group (weight-load amortization)
    nc = bacc.Bacc()
    dbg_d = None
    if stage < 4:
        dbg_shapes = {
            1: ([n_items // 2, 128, 2, SEQ], F32),
            2: ([n_items, 128, 512], BF16),
            3: ([n_items, 2, 128, 4, SEQ], BF16),
        }
        shp, dt = dbg_shapes[stage]
        dbg_d = nc.dram_tensor("dbg", shp, dt, kind="ExternalOutput")

    xT = nc.dram_tensor("xT", [n_items, D, SEQ], F32, kind="ExternalInput")
    wq_d = nc.dram_tensor("wq", [D, D], F32, kind="ExternalInput")
    wk_d = nc.dram_tensor("wk", [D, D], F32, kind="ExternalInput")
    wv_d = nc.dram_tensor("wv", [D, D], F32, kind="ExternalInput")
    bq_d = nc.dram_tensor("bq", [D], F32, kind="ExternalInput")
    bk_d = nc.dram_tensor("bk", [D], F32, kind="ExternalInput")
    mask_d = nc.dram_tensor("mask", [SEQ, SEQ], BF16, kind="ExternalInput")
    out_d = nc.dram_tensor(
        "out", [n_items, 33, 2, 2, 2, SEQ], F32, kind="ExternalOutput"
    )

    with tile.TileContext(nc) as tc, ExitStack() as ctx:
        const = ctx.enter_context(tc.tile_pool(name="const", bufs=1))
        xt_p = ctx.enter_context(tc.tile_pool(name="xt", bufs=4))
        qk_p = ctx.enter_context(tc.tile_pool(name="qk", bufs=8))
        v_p = ctx.enter_context(tc.tile_pool(name="vsb", bufs=4))
        e_p = ctx.enter_context(tc.tile_pool(name="e", bufs=4))
        em_p = ctx.enter_context(tc.tile_pool(name="em", bufs=4))
        osb_p = ctx.enter_context(tc.tile_pool(name="osb", bufs=3))
        # PSUM pools (8 banks total):
        pp_ps = ctx.enter_context(tc.tile_pool(name="pp", bufs=1, space="PSUM"))
        v_ps = ctx.enter_context(tc.tile_pool(name="vps", bufs=1, space="PSUM"))
        s0_ps = ctx.enter_context(tc.tile_pool(name="s0", bufs=1, space="PSUM"))
        s1_ps = ctx.enter_context(tc.tile_pool(name="s1", bufs=1, space="PSUM"))
        o_ps = ctx.enter_context(tc.tile_pool(name="ops", bufs=1, space="PSUM"))

        # ---- constants ----
        wq_sb = const.tile([128, 2, D], F32)
        wk_sb = const.tile([128, 2, D], F32)
        wv_sb = const.tile([128, 2, D], F32)
        for w_sb, w_d in ((wq_sb, wq_d), (wk_sb, wk_d), (wv_sb, wv_d)):
            for kc in range(2):
                nc.sync.dma_start(
                    out=w_sb[:, kc, :], in_=w_d[kc * 128:(kc + 1) * 128, :]
                )
        bq_sb = const.tile([128, 2], F32)
        bk_sb = const.tile([128, 2], F32)
        for b_sb, b_d in ((bq_sb, bq_d), (bk_sb, bk_d)):
            for dc in range(2):
                nc.sync.dma_start(
                    out=b_sb[:, dc:dc + 1],
                    in_=b_d[dc * 128:(dc + 1) * 128].rearrange("(a b) -> a b", b=1),
                )
        # replicated masks: mask0 [128, 4, 176] (sk 0:128), mask1 [48, 4, 176]
        mask0 = const.tile([128, 4, SEQ], BF16)
        mask1 = const.tile([48, 4, SEQ], BF16)
        for r in range(4):
            nc.sync.dma_start(out=mask0[:, r, :], in_=mask_d[0:128, :])
            nc.sync.dma_start(out=mask1[:, r, :], in_=mask_d[128:SEQ, :])

        def v_cols_ap(v_tile, rows):
            """Strided copy-target: [rows, 8, 32] at 33-col head pitch."""
            base = v_tile[0:rows, :]
            return bass.AP(
                tensor=base.tensor, offset=base.offset,
                ap=[base.ap[0], [33, 8], [1, 32]],
            )

        def v_ones_ap(v_tile, rows):
            base = v_tile[0:rows, :]
            return bass.AP(
                tensor=base.tensor, offset=base.offset + 32,
                ap=[base.ap[0], [33, 8], [1, 1]],
            )

        for g in range(n_items // G):
            # ---------- projections for G items ----------
            xt = []
            for kc in range(2):
                t = xt_p.tile([128, G, SEQ], F32, tag="xt")
                nc.sync.dma_start(
                    out=t,
                    in_=xT[g * G:(g + 1) * G, kc * 128:(kc + 1) * 128, :]
                    .rearrange("g p s -> p g s"),
                )
                xt.append(t)
            qt, kt = [], []
            for w_sb, b_sb, dst in ((wq_sb, bq_sb, qt), (wk_sb, bk_sb, kt)):
                for dc in range(2):
                    ps = pp_ps.tile([128, G, SEQ], F32, tag="pp")
                    for kc in range(2):
                        nc.tensor.matmul(
                            out=ps,
                            lhsT=w_sb[:, kc, dc * 128:(dc + 1) * 128],
                            rhs=xt[kc].rearrange("p g s -> p (g s)"),
                            start=(kc == 0),
                            stop=(kc == 1),
                        )
                    sb = qk_p.tile([128, G, SEQ], F32, tag="qk")
                    nc.vector.tensor_scalar_add(
                        out=sb.rearrange("p g s -> p (g s)"),
                        in0=ps.rearrange("p g s -> p (g s)"),
                        scalar1=b_sb[:, dc:dc + 1],
                    )
                    dst.append(sb)
            if stage == 1:
                nc.sync.dma_start(out=dbg_d[g], in_=qt[0])
                continue

            for i_loc in range(G):
                item = g * G + i_loc
                # ---------- V (natural layout) ----------
                v_sb = []
                for sc, (s0, slen) in enumerate(((0, 128), (128, 48))):
                    ps = v_ps.tile([slen, D], F32, tag="vps")
                    for kc in range(2):
                        nc.tensor.matmul(
                            out=ps,
                            lhsT=xt[kc][:, i_loc, s0:s0 + slen],
                            rhs=wv_sb[:, kc, :],
                            start=(kc == 0),
                            stop=(kc == 1),
                        )
                    vt = v_p.tile([slen, 264], BF16, tag=f"v{sc}")
                    nc.gpsimd.memset(v_ones_ap(vt, slen), 1.0)
                    nc.vector.tensor_copy(
                        out=v_cols_ap(vt, slen),
                        in_=ps.rearrange("p (h c) -> p h c", h=8),
                    )
                    v_sb.append(vt)
                if stage == 2:
                    nc.sync.dma_start(out=dbg_d[item], in_=v_sb[0])
                    continue

                # ---------- scores + exp + mask, per head-group c ----------
                em0s, em1s = [], []
                for c in range(2):
                    sct = s0_ps.tile([128, 4, 512], F32, tag="s0")
                    for j in (j_list if j_list is not None else range(max_j)):
                        lhs_full = kt[c][32 * j:32 * (j + 1), i_loc, :]
                        rhs_q = qt[c][32 * j:32 * (j + 1), i_loc, :]
                        nc.tensor.matmul(
                            out=sct[:, j, 0:SEQ], lhsT=lhs_full[:, 0:128],
                            rhs=rhs_q, start=True, stop=True,
                            tile_position=(32 * j, 0),
                        )
                        nc.tensor.matmul(
                            out=sct[0:48, j, 256:256 + SEQ],
                            lhsT=lhs_full[:, 128:SEQ],
                            rhs=rhs_q, start=True, stop=True,
                            tile_position=(32 * j, 0),
                        )
                    e0 = e_p.tile([128, 4, SEQ], BF16, tag="e0")
                    e1 = e_p.tile([48, 4, SEQ], BF16, tag="e1")
                    if no_exp:
                        nc.vector.tensor_copy(out=e0, in_=sct[:, :, 0:SEQ])
                        nc.vector.tensor_copy(
                            out=e1, in_=sct[0:48, :, 256:256 + SEQ])
                    else:
                        nc.scalar.activation(
                            out=e0, in_=sct[:, :, 0:SEQ],
                            func=mybir.ActivationFunctionType.Exp,
                        )
                        nc.scalar.activation(
                            out=e1, in_=sct[0:48, :, 256:256 + SEQ],
                            func=mybir.ActivationFunctionType.Exp,
                        )
                    em0 = em_p.tile([128, 4, SEQ], BF16, tag="em0")
                    em1 = em_p.tile([48, 4, SEQ], BF16, tag="em1")
                    nc.vector.tensor_mul(out=em0, in0=e0, in1=mask0)
                    if no_gps:
                        nc.vector.tensor_mul(out=em1, in0=e1, in1=mask1)
                    else:
                        nc.gpsimd.tensor_mul(out=em1, in0=e1, in1=mask1)
                    em0s.append(em0)
                    em1s.append(em1)
                if stage == 3:
                    for c in range(2):
                        nc.sync.dma_start(out=dbg_d[item, c], in_=em0s[c])
                    continue

                # ---------- attn @ V_aug ----------
                # wave w: heads 4w+u; u -> (bank u//2, cols 256*(u%2)),
                # all outputs at rows 0:33 (same col group -> serialized)
                osb = osb_p.tile([128, 2, 2, 2, SEQ], F32, tag="osb")
                for w in range(2):
                    op = o_ps.tile([128, 2, 512], F32, tag="ops")
                    for u in range(4):
                        h = 4 * w + u
                        c, j = h // 4, h % 4
                        out_ap = op[0:33, u // 2, 256 * (u % 2):256 * (u % 2) + SEQ]
                        nc.tensor.matmul(
                            out=out_ap, lhsT=v_sb[0][:, 33 * h:33 * h + 33],
                            rhs=em0s[c][:, j, :], start=True, stop=False,
                        )
                        nc.tensor.matmul(
                            out=out_ap, lhsT=v_sb[1][:, 33 * h:33 * h + 33],
                            rhs=em1s[c][:, j, :], start=False, stop=True,
                        )
                    copy_in = bass.AP(
                        tensor=op.tensor, offset=op.offset,
                        ap=[[op.ap[0][0], 33], [512, 2], [256, 2], [1, SEQ]],
                    )

                    if w == 0:
                        nc.vector.tensor_copy(
                            out=osb[0:33, w, :, :, :], in_=copy_in)
                    else:
                        nc.scalar.activation(
                            out=osb[0:33, w, :, :, :], in_=copy_in,
                            func=mybir.ActivationFunctionType.Copy,
                        )
                nc.sync.dma_start(
                    out=out_d[item], in_=osb[0:33, :, :, :, :],
                )
    return nc


def kernel(**inputs) -> np.ndarray:
    x = np.asarray(inputs["x"], dtype=np.float32)
    Wq = np.asarray(inputs["Wq"], dtype=np.float32)
    bq = np.asarray(inputs["bq"], dtype=np.float32)
    Wk = np.asarray(inputs["Wk"], dtype=np.float32)
    bk = np.asarray(inputs["bk"], dtype=np.float32)
    Wv = np.asarray(inputs["Wv"], dtype=np.float32)
    bv = np.asarray(inputs["bv"], dtype=np.float32)
    mask = np.asarray(inputs["mask"], dtype=np.float32)

    import ml_dtypes

    B = x.shape[0]
    n_items = B // N_CORES
    scale = 1.0 / math.sqrt(HD)
    xT = np.ascontiguousarray(x.transpose(0, 2, 1))
    wqs = np.ascontiguousarray(Wq * scale)
    bqs = np.ascontiguousarray(bq * scale)
    mask_bf = mask.astype(ml_dtypes.bfloat16)

    nc = build_nc(n_items)
    nc.finalize()
    in_maps = []
    for c in range(N_CORES):
        in_maps.append({
            "xT": xT[c * n_items:(c + 1) * n_items],
            "wq": wqs, "wk": np.ascontiguousarray(Wk),
            "wv": np.ascontiguousarray(Wv),
            "bq": bqs, "bk": np.ascontiguousarray(bk),
            "mask": mask_bf,
        })
    from concourse.bass_utils import run_bass_kernel_spmd

    res = run_bass_kernel_spmd(nc, in_maps, list(range(N_CORES)))
    ot = np.concatenate([np.asarray(res.results[c]["out"]) for c in range(N_CORES)])
    # ot: [B, 33, w, b, cs, s]; head h = 4w + 2b + cs; row 32 = S
    num = ot[:, 0:32]                          # [B, d, w, b, cs, s]
    S = ot[:, 32:33]
    att = num / S
    # -> [B, s, w, b, cs, d]; flatten (w,b,cs,d) = h*32+d
    out = att.transpose(0, 5, 2, 3, 4, 1).reshape(B, SEQ, D)
    out = out + bv
    return np.ascontiguousarray(out.astype(np.float32))
